# revision 10
# baseline (speedup 1.0000x reference)
"""Cross-attention (softmax over queries) Trainium2 Bass kernel.

Shards batch B=8 across 8 NeuronCores (one batch element per core, no
collectives). Per core, for img/text [N=2048, D=1024]:

  Q = img @ Wq^T ; K = text @ Wk^T ; V = text @ Wv^T
  S = Q @ K^T                      # [n, m] logits, std ~ sqrt(D)
  alpha = softmax(S, axis=n)       # over QUERIES -> free-dim reduction
                                   # when S is laid out [m, n]
  out = alpha @ V ; feature = out + text

The K projection is folded into the host: A = Wq^T @ Wk (a weights-only
4 MiB precompute), so the device computes S = (img @ A) @ text^T — one fewer
projection matmul per core and textT feeds the S matmul directly as the
stationary operand.

Precision: logit path (U = img@A, S) uses float32r matmuls — fp32 bits streamed
through the PE in single-pass mode (1 cycle/row when the moving operand is
>=256 wide, vs 4 cycles/row for plain fp32). exp stays fp32. alpha and
V' = (1/Z)*V are fp16 (their errors enter the output linearly), which halves
SBUF/DMA and runs the second attention matmul at full rate.

Layouts: host pre-transposes imgT/textT = x^T [D, N] so the PE contracts
over d; weights fed as W^T. QT [k, n] stays SBUF resident; alpha [m, n]
round-trips DRAM in fp16; V' is computed per m-tile fused into the softmax
phase (reuses the textT block loaded for K^T). m-tiles are processed in
pairs so the K^T-block matmuls also get a 256-wide moving operand.
"""

import os
import sys

import numpy as np

for _p in ("/opt/trn_rl_repo",):
    if _p not in sys.path and os.path.isdir(_p):
        sys.path.insert(0, _p)

import concourse.mybir as mybir
from concourse import bacc
from concourse.bass_utils import run_bass_kernel_spmd
from concourse.tile import TileContext

F32 = mybir.dt.float32
F32R = mybir.dt.float32r
F16 = mybir.dt.float16
AX = mybir.AxisListType.X
AF = mybir.ActivationFunctionType
P = 128
USE_F32R = True
DT_L = F32R if USE_F32R else F32  # logit-path matmul storage dtype


def build_nc(N=2048, D=1024):
    """Build the single-core Bass program (SPMD across all cores)."""
    DT = D // P          # d/k tiles of 128
    NT = N // P          # n/m tiles of 128
    CH = min(512, N)     # S-matmul free chunk (psum bank)
    NC = N // CH
    SCH = min(256, N)    # P1 img moving chunk
    SNC = N // SCH
    WCH = max(P, D // 4)  # weight load chunk width
    KPC = WCH // P       # k-tiles per weight chunk
    VCH = min(512, D)    # v chunk (V' + out matmuls)
    VC = D // VCH
    MP = min(2, NT)      # m-tiles per pair
    PAIR = P * MP

    nc = bacc.Bacc(None)
    imgT = nc.declare_dram_parameter("imgT", [D, N], DT_L, isOutput=False)
    textT = nc.declare_dram_parameter("textT", [D, N], DT_L, isOutput=False)
    text_nat = nc.declare_dram_parameter("text_nat", [N, D], F32, isOutput=False)
    a_d = nc.declare_dram_parameter("A", [D, D], DT_L, isOutput=False)
    wvT_d = nc.declare_dram_parameter("wvT", [D, D], DT_L, isOutput=False)
    out_d = nc.declare_dram_parameter("out", [N, D], F32, isOutput=True)
    feat_d = nc.declare_dram_parameter("feat", [N, D], F32, isOutput=True)

    with TileContext(nc) as tc:
        with (
            tc.tile_pool(name="big", bufs=1) as big,
            tc.tile_pool(name="small", bufs=4) as small,
            tc.tile_pool(name="pmm", bufs=2, space="PSUM") as pmm,
        ):
            qt = big.tile([P, DT, N], DT_L, tag="qt")      # QT[k%P, k//P, n]
            vp = big.tile([P, NT, D], F16, tag="vp")      # V'[m%P, m//P, v]
            alpha = big.tile([P, NT, N], F16, tag="alpha")  # exp(S-mx)[m%P, m//P, n]
            invz = big.tile([P, NT], F32, tag="invz")

            def load_w(pool, wdram):
                chunks = []
                for h in range(D // WCH):
                    w = pool.tile([P, DT, WCH], DT_L, tag="w")
                    nc.sync.dma_start(
                        out=w,
                        in_=wdram[:, h * WCH:(h + 1) * WCH].rearrange(
                            "(dt p) k -> p dt k", p=P
                        ),
                    )
                    chunks.append(w)
                return chunks

            # ---- P1: QT[k, n] = Wq @ imgT, fp32 bits, SBUF resident ----
            with (
                tc.tile_pool(name="p1w", bufs=2) as wpool,
                tc.tile_pool(name="p1s", bufs=2) as spool,
            ):
                wq = load_w(wpool, a_d)
                for ncl in range(SNC):
                    ib = spool.tile([P, DT, SCH], DT_L, tag="xblk")
                    nc.sync.dma_start(
                        out=ib,
                        in_=imgT[:, ncl * SCH:(ncl + 1) * SCH].rearrange(
                            "(dt p) n -> p dt n", p=P
                        ),
                    )
                    for kt in range(DT):
                        w, koff = wq[kt // KPC], (kt % KPC) * P
                        ps = pmm.tile([P, SCH], F32, tag="mm")
                        for d in range(DT):
                            nc.tensor.matmul(
                                ps, lhsT=w[:, d, koff:koff + P],
                                rhs=ib[:, d, :],
                                start=(d == 0), stop=(d == DT - 1),
                            )
                        nc.scalar.copy(
                            out=qt[:, kt, ncl * SCH:(ncl + 1) * SCH], in_=ps
                        )

            # ---- P2: per m-pair: KT -> per m-tile: S -> softmax -> V' ----
            with (
                tc.tile_pool(name="p2wv", bufs=1) as wvpool,
                tc.tile_pool(name="p2s", bufs=2) as spool,
                tc.tile_pool(name="pst", bufs=1, space="PSUM") as pst,
            ):
                # Wv in fp16 (V path errors are linear in the output)
                wv16 = wvpool.tile([P, DT, D], F16, tag="wv16")
                for c in range(D // PAIR):
                    ws = spool.tile([P, DT, PAIR], DT_L, tag="tb2")
                    nc.sync.dma_start(
                        out=ws,
                        in_=wvT_d[:, c * PAIR:(c + 1) * PAIR].rearrange(
                            "(dt p) v -> p dt v", p=P
                        ),
                    )
                    nc.vector.tensor_copy(
                        out=wv16[:, :, c * PAIR:(c + 1) * PAIR],
                        in_=ws.bitcast(F32) if USE_F32R else ws,
                    )
                for pr in range(NT // MP):
                    tb2 = spool.tile([P, DT, PAIR], DT_L, tag="tb2")
                    nc.sync.dma_start(
                        out=tb2,
                        in_=textT[:, pr * PAIR:(pr + 1) * PAIR].rearrange(
                            "(dt p) m -> p dt m", p=P
                        ),
                    )
                    tb16 = spool.tile([P, DT, PAIR], F16, tag="tb16")
                    nc.vector.tensor_copy(out=tb16, in_=tb2.bitcast(F32) if USE_F32R else tb2)
                    for mi in range(MP):
                        mt = pr * MP + mi
                        mo = mi * P
                        # S chunks [m, n] live in PSUM; softmax over free n
                        pss = []
                        for ncl in range(NC):
                            ps = pst.tile([P, CH], F32, tag=f"s{ncl}")
                            for kt in range(DT):
                                nc.tensor.matmul(
                                    ps, lhsT=tb2[:, kt, mo:mo + P],
                                    rhs=qt[:, kt, ncl * CH:(ncl + 1) * CH],
                                    start=(kt == 0), stop=(kt == DT - 1),
                                )
                            pss.append(ps)
                        mxc = small.tile([P, NC], F32, tag="mxc")
                        for ncl in range(NC):
                            nc.vector.reduce_max(
                                mxc[:, ncl:ncl + 1], pss[ncl], axis=AX
                            )
                        nmx = small.tile([P, 1], F32, tag="nmx")
                        nc.vector.reduce_max(nmx, mxc, axis=AX, negate=True)
                        zp = small.tile([P, NC], F32, tag="zp")
                        for ncl in range(NC):
                            nc.scalar.activation(
                                out=alpha[:, mt, ncl * CH:(ncl + 1) * CH],
                                in_=pss[ncl], func=AF.Exp,
                                bias=nmx, scale=1.0,
                                accum_out=zp[:, ncl:ncl + 1],
                            )
                        z = small.tile([P, 1], F32, tag="z")
                        nc.vector.reduce_sum(z, zp, axis=AX)
                        nc.vector.reciprocal(invz[:, mt:mt + 1], z)
                        # V' rows: (1/Z[m]) * (text @ Wv^T)[m, :]
                        for vc in range(VC):
                            pv = pmm.tile([P, VCH], F32, tag="mm")
                            for d in range(DT):
                                nc.tensor.matmul(
                                    pv, lhsT=tb16[:, d, mo:mo + P],
                                    rhs=wv16[:, d, vc * VCH:(vc + 1) * VCH],
                                    start=(d == 0), stop=(d == DT - 1),
                                )
                            nc.scalar.mul(
                                out=vp[:, mt, vc * VCH:(vc + 1) * VCH],
                                in_=pv, mul=invz[:, mt:mt + 1],
                            )

            # ---- P3: out[n, v] = sum_m alpha[m, n] V'[m, v]; feature ----
            with tc.tile_pool(name="p3s", bufs=2) as spool:
                for nt in range(NT):
                    for vc in range(VC):
                        po = pmm.tile([P, VCH], F32, tag="mm")
                        for mt in range(NT):
                            nc.tensor.matmul(
                                po, lhsT=alpha[:, mt, nt * P:(nt + 1) * P],
                                rhs=vp[:, mt, vc * VCH:(vc + 1) * VCH],
                                start=(mt == 0), stop=(mt == NT - 1),
                            )
                        ob = spool.tile([P, VCH], F32, tag="ob")
                        nc.scalar.copy(out=ob, in_=po)
                        nc.sync.dma_start(
                            out=out_d[
                                nt * P:(nt + 1) * P, vc * VCH:(vc + 1) * VCH
                            ],
                            in_=ob,
                        )
                        tn = spool.tile([P, VCH], F32, tag="tn")
                        nc.sync.dma_start(
                            out=tn,
                            in_=text_nat[
                                nt * P:(nt + 1) * P, vc * VCH:(vc + 1) * VCH
                            ],
                        )
                        fb = spool.tile([P, VCH], F32, tag="fb")
                        nc.vector.tensor_add(fb, ob, tn)
                        nc.sync.dma_start(
                            out=feat_d[
                                nt * P:(nt + 1) * P, vc * VCH:(vc + 1) * VCH
                            ],
                            in_=fb,
                        )

    nc.compile()
    return nc


_NC_CACHE = {}
LAST_RESULT = None  # BassKernelResults of the most recent run (for profiling)


def _get_nc(N, D):
    key = (N, D)
    if key not in _NC_CACHE:
        _NC_CACHE[key] = build_nc(N, D)
    return _NC_CACHE[key]


def kernel(img, text, Wq, Wk, Wv):
    img = np.asarray(img, dtype=np.float32)
    text = np.asarray(text, dtype=np.float32)
    Wq = np.asarray(Wq, dtype=np.float32)
    Wk = np.asarray(Wk, dtype=np.float32)
    Wv = np.asarray(Wv, dtype=np.float32)
    B, N, D = img.shape

    nc = _get_nc(N, D)

    imgT = np.ascontiguousarray(np.swapaxes(img, 1, 2))
    textT = np.ascontiguousarray(np.swapaxes(text, 1, 2))
    A = np.ascontiguousarray(Wq.T @ Wk)
    wvT = np.ascontiguousarray(Wv.T)

    in_maps = [
        {
            "imgT": imgT[b],
            "textT": textT[b],
            "text_nat": np.ascontiguousarray(text[b]),
            "A": A,
            "wvT": wvT,
        }
        for b in range(B)
    ]
    global LAST_RESULT
    LAST_RESULT = run_bass_kernel_spmd(nc, in_maps, list(range(B)))
    res = LAST_RESULT.results
    out = np.stack([r["out"] for r in res])
    feat = np.stack([r["feat"] for r in res])
    return out, feat


# revision 12
# speedup vs baseline: 1.3823x; 1.3823x over previous
"""Cross-attention (softmax over queries) Trainium2 Bass kernel.

Shards batch B=8 across 8 NeuronCores (one batch element per core, no
collectives). Per core, for img/text [N=2048, D=1024]:

  Q = img @ Wq^T ; K = text @ Wk^T ; V = text @ Wv^T
  S = Q @ K^T                      # [n, m] logits, std ~ sqrt(D)
  alpha = softmax(S, axis=n)       # over QUERIES -> free-dim reduction
                                   # when S is laid out [m, n]
  out = alpha @ V ; feature = out + text

The K projection is folded into the host: A = Wq^T @ Wk (a weights-only
4 MiB precompute), so the device computes S = (img @ A) @ text^T — one fewer
projection matmul per core and textT feeds the S matmul directly as the
stationary operand.

Precision: logit path (U = img@A, S) uses float32r matmuls — fp32 bits streamed
through the PE in single-pass mode (1 cycle/row when the moving operand is
>=256 wide, vs 4 cycles/row for plain fp32). exp stays fp32. alpha and
V' = (1/Z)*V are fp16 (their errors enter the output linearly), which halves
SBUF/DMA and runs the second attention matmul at full rate.

Layouts: host pre-transposes imgT/textT = x^T [D, N] so the PE contracts
over d; weights fed as W^T. QT [k, n] stays SBUF resident; alpha [m, n]
round-trips DRAM in fp16; V' is computed per m-tile fused into the softmax
phase (reuses the textT block loaded for K^T). m-tiles are processed in
pairs so the K^T-block matmuls also get a 256-wide moving operand.
"""

import os
import sys

import numpy as np

for _p in ("/opt/trn_rl_repo",):
    if _p not in sys.path and os.path.isdir(_p):
        sys.path.insert(0, _p)

import concourse.mybir as mybir
from concourse import bacc
from concourse.bass_utils import run_bass_kernel_spmd
from concourse.tile import TileContext

F32 = mybir.dt.float32
F32R = mybir.dt.float32r
F16 = mybir.dt.float16
AX = mybir.AxisListType.X
AF = mybir.ActivationFunctionType
P = 128
USE_F32R = True
DT_L = F32R if USE_F32R else F32  # logit-path matmul storage dtype


def build_nc(N=2048, D=1024):
    """Build the single-core Bass program (SPMD across all cores)."""
    DT = D // P          # d/k tiles of 128
    NT = N // P          # n/m tiles of 128
    CH = min(512, N)     # moving chunk for QT matmul
    NC = N // CH
    HK = DT // 2         # k-tiles per weight half
    WH = D // 2          # weight half width
    VCH = min(512, D)    # v chunk (V' + out matmuls)
    VC = D // VCH
    MP = min(2, NT)      # m-tiles per pair
    PAIR = P * MP

    nc = bacc.Bacc(None)
    imgT = nc.declare_dram_parameter("imgT", [D, N], DT_L, isOutput=False)
    textT = nc.declare_dram_parameter("textT", [D, N], DT_L, isOutput=False)
    text_nat = nc.declare_dram_parameter("text_nat", [N, D], F32, isOutput=False)
    a_d = nc.declare_dram_parameter("A", [D, D], DT_L, isOutput=False)
    wvT_d = nc.declare_dram_parameter("wvT", [D, D], F32, isOutput=False)
    out_d = nc.declare_dram_parameter("out", [N, D], F32, isOutput=True)
    feat_d = nc.declare_dram_parameter("feat", [N, D], F32, isOutput=True)
    alpha_d = nc.dram_tensor("alpha_tmp", [N, N], F16)

    with TileContext(nc) as tc:
        with (
            tc.tile_pool(name="big", bufs=1) as big,
            tc.tile_pool(name="small", bufs=4) as small,
            tc.tile_pool(name="pmm", bufs=2, space="PSUM") as pmm,
        ):
            qt = big.tile([P, DT, N], DT_L, tag="qt")      # QT[k%P, k//P, n]
            vp = big.tile([P, NT, D], F16, tag="vp")      # V'[m%P, m//P, v]
            invz = big.tile([P, NT], F32, tag="invz")

            def load_w(pool, wdram):
                halves = []
                for h in range(2):
                    w = pool.tile([P, DT, WH], DT_L, tag="w")
                    nc.sync.dma_start(
                        out=w,
                        in_=wdram[:, h * WH:(h + 1) * WH].rearrange(
                            "(dt p) k -> p dt k", p=P
                        ),
                    )
                    halves.append(w)
                return halves

            # ---- P1: QT[k, n] = Wq @ imgT, fp32 bits, SBUF resident ----
            with (
                tc.tile_pool(name="p1w", bufs=2) as wpool,
                tc.tile_pool(name="p1s", bufs=2) as spool,
            ):
                wq = load_w(wpool, a_d)
                for ncl in range(NC):
                    ib = spool.tile([P, DT, CH], DT_L, tag="xblk")
                    nc.sync.dma_start(
                        out=ib,
                        in_=imgT[:, ncl * CH:(ncl + 1) * CH].rearrange(
                            "(dt p) n -> p dt n", p=P
                        ),
                    )
                    for kt in range(DT):
                        w, koff = wq[kt // HK], (kt % HK) * P
                        ps = pmm.tile([P, CH], F32, tag="mm")
                        for d in range(DT):
                            nc.tensor.matmul(
                                ps, lhsT=w[:, d, koff:koff + P],
                                rhs=ib[:, d, :],
                                start=(d == 0), stop=(d == DT - 1),
                            )
                        nc.scalar.copy(
                            out=qt[:, kt, ncl * CH:(ncl + 1) * CH], in_=ps
                        )

            # ---- P2: per m-pair: KT -> per m-tile: S -> softmax -> V' ----
            with (
                tc.tile_pool(name="p2wv", bufs=1) as wvpool,
                tc.tile_pool(name="p2s", bufs=2) as spool,
                tc.tile_pool(name="p2a", bufs=4) as apool,
                tc.tile_pool(name="pst", bufs=1, space="PSUM") as pst,
            ):
                # Wv in fp16 (V path errors are linear in the output)
                wv16 = wvpool.tile([P, DT, D], F16, tag="wv16")
                for c in range(D // PAIR):
                    ws = spool.tile([P, DT, PAIR], F32, tag="wvs")
                    nc.sync.dma_start(
                        out=ws,
                        in_=wvT_d[:, c * PAIR:(c + 1) * PAIR].rearrange(
                            "(dt p) v -> p dt v", p=P
                        ),
                    )
                    nc.vector.tensor_copy(
                        out=wv16[:, :, c * PAIR:(c + 1) * PAIR], in_=ws
                    )
                for pr in range(NT // MP):
                    tb2 = spool.tile([P, DT, PAIR], DT_L, tag="tb2")
                    nc.sync.dma_start(
                        out=tb2,
                        in_=textT[:, pr * PAIR:(pr + 1) * PAIR].rearrange(
                            "(dt p) m -> p dt m", p=P
                        ),
                    )
                    tb16 = spool.tile([P, DT, PAIR], F16, tag="tb16")
                    nc.vector.tensor_copy(out=tb16, in_=tb2.bitcast(F32) if USE_F32R else tb2)
                    for mi in range(MP):
                        mt = pr * MP + mi
                        mo = mi * P
                        # S chunks [m, n] live in PSUM; softmax over free n
                        pss = []
                        for ncl in range(NC):
                            ps = pst.tile([P, CH], F32, tag=f"s{ncl}")
                            for kt in range(DT):
                                nc.tensor.matmul(
                                    ps, lhsT=tb2[:, kt, mo:mo + P],
                                    rhs=qt[:, kt, ncl * CH:(ncl + 1) * CH],
                                    start=(kt == 0), stop=(kt == DT - 1),
                                )
                            pss.append(ps)
                        mxc = small.tile([P, NC], F32, tag="mxc")
                        for ncl in range(NC):
                            nc.vector.reduce_max(
                                mxc[:, ncl:ncl + 1], pss[ncl], axis=AX
                            )
                        nmx = small.tile([P, 1], F32, tag="nmx")
                        nc.vector.reduce_max(nmx, mxc, axis=AX, negate=True)
                        zp = small.tile([P, NC], F32, tag="zp")
                        for ncl in range(NC):
                            ae = apool.tile([P, CH], F16, tag="ae")
                            nc.scalar.activation(
                                out=ae, in_=pss[ncl], func=AF.Exp,
                                bias=nmx, scale=1.0,
                                accum_out=zp[:, ncl:ncl + 1],
                            )
                            nc.sync.dma_start(
                                out=alpha_d[
                                    mt * P:(mt + 1) * P,
                                    ncl * CH:(ncl + 1) * CH,
                                ],
                                in_=ae,
                            )
                        z = small.tile([P, 1], F32, tag="z")
                        nc.vector.reduce_sum(z, zp, axis=AX)
                        nc.vector.reciprocal(invz[:, mt:mt + 1], z)
                        # V' rows: (1/Z[m]) * (text @ Wv^T)[m, :]
                        for vc in range(VC):
                            pv = pmm.tile([P, VCH], F32, tag="mm")
                            for d in range(DT):
                                nc.tensor.matmul(
                                    pv, lhsT=tb16[:, d, mo:mo + P],
                                    rhs=wv16[:, d, vc * VCH:(vc + 1) * VCH],
                                    start=(d == 0), stop=(d == DT - 1),
                                )
                            nc.scalar.mul(
                                out=vp[:, mt, vc * VCH:(vc + 1) * VCH],
                                in_=pv, mul=invz[:, mt:mt + 1],
                            )

            # ---- P3: out[n, v] = sum_m alpha[m, n] V'[m, v]; feature ----
            with tc.tile_pool(name="p3s", bufs=2) as spool:
                for nt in range(NT):
                    ab = spool.tile([P, NT, P], F16, tag="ab")
                    nc.sync.dma_start(
                        out=ab,
                        in_=alpha_d[:, nt * P:(nt + 1) * P].rearrange(
                            "(mt p) n -> p mt n", p=P
                        ),
                    )
                    for vc in range(VC):
                        po = pmm.tile([P, VCH], F32, tag="mm")
                        for mt in range(NT):
                            nc.tensor.matmul(
                                po, lhsT=ab[:, mt, :],
                                rhs=vp[:, mt, vc * VCH:(vc + 1) * VCH],
                                start=(mt == 0), stop=(mt == NT - 1),
                            )
                        ob = spool.tile([P, VCH], F32, tag="ob")
                        nc.scalar.copy(out=ob, in_=po)
                        nc.sync.dma_start(
                            out=out_d[
                                nt * P:(nt + 1) * P, vc * VCH:(vc + 1) * VCH
                            ],
                            in_=ob,
                        )
                        tn = spool.tile([P, VCH], F32, tag="tn")
                        nc.sync.dma_start(
                            out=tn,
                            in_=text_nat[
                                nt * P:(nt + 1) * P, vc * VCH:(vc + 1) * VCH
                            ],
                        )
                        fb = spool.tile([P, VCH], F32, tag="fb")
                        nc.vector.tensor_add(fb, ob, tn)
                        nc.sync.dma_start(
                            out=feat_d[
                                nt * P:(nt + 1) * P, vc * VCH:(vc + 1) * VCH
                            ],
                            in_=fb,
                        )

    nc.compile()
    return nc


_NC_CACHE = {}
LAST_RESULT = None  # BassKernelResults of the most recent run (for profiling)


def _get_nc(N, D):
    key = (N, D)
    if key not in _NC_CACHE:
        _NC_CACHE[key] = build_nc(N, D)
    return _NC_CACHE[key]


def kernel(img, text, Wq, Wk, Wv):
    img = np.asarray(img, dtype=np.float32)
    text = np.asarray(text, dtype=np.float32)
    Wq = np.asarray(Wq, dtype=np.float32)
    Wk = np.asarray(Wk, dtype=np.float32)
    Wv = np.asarray(Wv, dtype=np.float32)
    B, N, D = img.shape

    nc = _get_nc(N, D)

    imgT = np.ascontiguousarray(np.swapaxes(img, 1, 2))
    textT = np.ascontiguousarray(np.swapaxes(text, 1, 2))
    A = np.ascontiguousarray(Wq.T @ Wk)
    wvT = np.ascontiguousarray(Wv.T)

    in_maps = [
        {
            "imgT": imgT[b],
            "textT": textT[b],
            "text_nat": np.ascontiguousarray(text[b]),
            "A": A,
            "wvT": wvT,
        }
        for b in range(B)
    ]
    global LAST_RESULT
    LAST_RESULT = run_bass_kernel_spmd(nc, in_maps, list(range(B)))
    res = LAST_RESULT.results
    out = np.stack([r["out"] for r in res])
    feat = np.stack([r["feat"] for r in res])
    return out, feat
